# revision 5
# baseline (speedup 1.0000x reference)
"""Trainium2 Bass kernel for nn_GNN_37615323579234 (gnn_message_passing).

Math (reference, N=8192, D=64, 4 layers; layer-3 A@H products are dead code):
    l=0..3:  H_cl = relu(X1@w1+b1) + relu(X2@w2+b2);  H_ue = relu(Xue@w3+b3)
             X1 = A_cl@H_cl;  X2 = A_ue@H_ue;  Xue = A_ue@H_cl
    out = relu(colsum(H_cl3) @ Qw1 + Qb1) @ Qw2 + Qb2      # [1,1]

Sharding: row-shard A_cl/A_ue over 8 cores (1024 rows each).  Host feeds each
core its A row-block TRANSPOSED ([8192,1024] contiguous) so the contraction dim
lands on SBUF partitions with line-rate DMA.  Big matmuls compute the output
TRANSPOSED: out^T[m,n] with stationary = H k-tile (natural layout), moving =
A^T k-tile.  H_ue|H_cl are interleaved per k-tile in one SBUF buffer so the
fused A_ue pass uses a single [128,128] stationary.  Biases are folded into
the small matmuls via an appended ones-row (stationary) / bias-row (weights).
Between layers: AllGather of the updated H blocks (DRAM bounce), AllReduce for
the final pooled vector.
"""

import os
import sys

for _p in ("/opt/trn_rl_repo", "/root/.axon_site/_ro/trn_rl_repo"):
    if os.path.isdir(_p) and _p not in sys.path:
        sys.path.insert(0, _p)

import numpy as np

N = 8192
D = 64
M = 8          # cores
R = N // M     # 1024 rows per core
P = 128        # partitions
KT = N // P    # 64 k-tiles
JT = R // P    # 8 row-tiles per core

F32 = None     # set after mybir import

LAST_EXEC_NS = None
LAST_PROFILE = None

_CACHED = None  # compile once per process


def _T(tc, shape, dtype, name):
    t, _free = tc.tile(shape, dtype, name=name)
    return t



def _build_module():
    import concourse.bacc as bacc
    import concourse.mybir as mybir
    from concourse import tile

    f32 = mybir.dt.float32
    RELU = mybir.ActivationFunctionType.Relu
    ADD = mybir.AluOpType.add
    BYPASS = mybir.AluOpType.bypass

    nc = bacc.Bacc(
        "TRN2",
        target_bir_lowering=False,
        debug=False,
        enable_asserts=False,
        num_devices=M,
    )

    # ---- I/O -------------------------------------------------------------
    AclT = nc.dram_tensor("AclT", [N, R], f32, kind="ExternalInput")
    AueT = nc.dram_tensor("AueT", [N, R], f32, kind="ExternalInput")
    X1T_d = nc.dram_tensor("X1T", [3, N], f32, kind="ExternalInput")
    X2T_d = nc.dram_tensor("X2T", [3, N], f32, kind="ExternalInput")
    XueT_d = nc.dram_tensor("XueT", [3, N], f32, kind="ExternalInput")
    w10_d = nc.dram_tensor("w10", [3, D], f32, kind="ExternalInput")
    w20_d = nc.dram_tensor("w20", [3, D], f32, kind="ExternalInput")
    w30_d = nc.dram_tensor("w30", [3, D], f32, kind="ExternalInput")
    w1x_d = nc.dram_tensor("w1x", [D + 1, 3, D], f32, kind="ExternalInput")
    w2x_d = nc.dram_tensor("w2x", [D + 1, 3, D], f32, kind="ExternalInput")
    w3x_d = nc.dram_tensor("w3x", [D + 1, 3, D], f32, kind="ExternalInput")
    q1x_d = nc.dram_tensor("q1x", [D + 1, D], f32, kind="ExternalInput")
    q2x_d = nc.dram_tensor("q2x", [D + 1, 1], f32, kind="ExternalInput")
    out_d = nc.dram_tensor("out", [1, 1], f32, kind="ExternalOutput")

    # internal DRAM for collectives
    Lg = nc.dram_tensor("Lg", [JT, P, 2 * D], f32)
    Gg = nc.dram_tensor("Gg", [KT, P, 2 * D], f32, addr_space="Shared")
    prd_l = nc.dram_tensor("prd_l", [D, 1], f32)
    prd_s = nc.dram_tensor("prd_s", [D, 1], f32, addr_space="Shared")

    groups = [list(range(M))]

    with tile.TileContext(nc) as tc, tc.tile_pool(name="persist", bufs=1) as pp:
        # persistent SBUF state
        Hbuf = pp.tile([P, KT, 2 * D], f32, tag="Hbuf")  # [:, k, 0:64]=H_ue, 64:128=H_cl
        w10 = pp.tile([3, D], f32, tag="w10s")
        w20 = pp.tile([3, D], f32, tag="w20s")
        w30 = pp.tile([3, D], f32, tag="w30s")
        w1x = pp.tile([D + 1, 3, D], f32, tag="w1xs")
        w2x = pp.tile([D + 1, 3, D], f32, tag="w2xs")
        w3x = pp.tile([D + 1, 3, D], f32, tag="w3xs")
        q1x = pp.tile([D + 1, D], f32, tag="q1xs")
        q2x = pp.tile([D + 1, 1], f32, tag="q2xs")
        ones_mv = pp.tile([P, 1], f32, tag="ones_mv")

        nc.sync.dma_start(out=w10[:], in_=w10_d[:])
        nc.sync.dma_start(out=w20[:], in_=w20_d[:])
        nc.sync.dma_start(out=w30[:], in_=w30_d[:])
        nc.sync.dma_start(out=w1x[:], in_=w1x_d[:])
        nc.sync.dma_start(out=w2x[:], in_=w2x_d[:])
        nc.sync.dma_start(out=w3x[:], in_=w3x_d[:])
        nc.sync.dma_start(out=q1x[:], in_=q1x_d[:])
        nc.sync.dma_start(out=q2x[:], in_=q2x_d[:])
        nc.gpsimd.memset(ones_mv[:], 1.0)

        with (
            tc.tile_pool(name="pa", bufs=6) as pa,
            tc.tile_pool(name="pb", bufs=6) as pb,
            tc.tile_pool(name="ps", bufs=1, space="PSUM") as ps,
            tc.tile_pool(name="sbE", bufs=2) as sbE,
            tc.tile_pool(name="pX", bufs=2) as pX,
        ):
            # ---- layer 0: full H0 for all N rows, interleaved into Hbuf --
            for g in range(8):
                gsl = slice(g * R, (g + 1) * R)
                x1c = pX.tile([3, R], f32, tag="x1c")
                x2c = pX.tile([3, R], f32, tag="x2c")
                xuc = pX.tile([3, R], f32, tag="xuc")
                nc.sync.dma_start(out=x1c[:], in_=X1T_d[:, gsl])
                nc.sync.dma_start(out=x2c[:], in_=X2T_d[:, gsl])
                nc.sync.dma_start(out=xuc[:], in_=XueT_d[:, gsl])
                pue = ps.tile([P, 8, D], f32, tag="pnue")
                pc1 = ps.tile([P, 8, D], f32, tag="pn1")
                pc2 = ps.tile([P, 8, D], f32, tag="pn2")
                for jj in range(8):
                    sl = slice(jj * P, (jj + 1) * P)
                    nc.tensor.matmul(pue[:, jj, :], xuc[:, sl], w30[:], start=True, stop=True)
                    nc.tensor.matmul(pc1[:, jj, :], x1c[:, sl], w10[:], start=True, stop=True)
                    nc.tensor.matmul(pc2[:, jj, :], x2c[:, sl], w20[:], start=True, stop=True)
                jsl = slice(g * 8, (g + 1) * 8)
                t1 = sbE.tile([P, 8, D], f32, tag="t1")
                t2 = sbE.tile([P, 8, D], f32, tag="t2")
                nc.scalar.activation(Hbuf[:, jsl, 0:D], pue[:], RELU)
                nc.scalar.activation(t1[:], pc1[:], RELU)
                nc.scalar.activation(t2[:], pc2[:], RELU)
                nc.vector.tensor_tensor(Hbuf[:, jsl, D : 2 * D], t1[:], t2[:], ADD)

            # ---- main layers ---------------------------------------------
            for l in range(3):
                last = l == 2
                mue = P if not last else D  # ue-pass stationary width
                Pcl0 = ps.tile([D, 512], f32, tag="acc_cl0")
                Pcl1 = ps.tile([D, 512], f32, tag="acc_cl1")
                Pue0 = ps.tile([mue, 512], f32, tag="acc_ue0")
                Pue1 = ps.tile([mue, 512], f32, tag="acc_ue1")
                for k in range(KT):
                    ksl = slice(k * P, (k + 1) * P)
                    at = pa.tile([P, R], f32, tag="acl")
                    bt = pb.tile([P, R], f32, tag="aue")
                    nc.sync.dma_start(out=at[:], in_=AclT[ksl, :])
                    nc.sync.dma_start(out=bt[:], in_=AueT[ksl, :])
                    st_cl = Hbuf[:, k, D : 2 * D]
                    st_ue = Hbuf[:, k, 0:mue]
                    s, e = k == 0, k == KT - 1
                    nc.tensor.matmul(Pcl0[:], st_cl, at[:, 0:512], start=s, stop=e)
                    nc.tensor.matmul(Pcl1[:], st_cl, at[:, 512:1024], start=s, stop=e)
                    nc.tensor.matmul(Pue0[:], st_ue, bt[:, 0:512], start=s, stop=e)
                    nc.tensor.matmul(Pue1[:], st_ue, bt[:, 512:1024], start=s, stop=e)

                # epilogue: X^T blocks -> next-layer H for this core's rows
                XT1 = sbE.tile([D + 1, R], f32, tag="xt1")
                XT2 = sbE.tile([D + 1, R], f32, tag="xt2")
                nc.vector.tensor_copy(XT1[0:D, 0:512], Pcl0[:])
                nc.vector.tensor_copy(XT1[0:D, 512:1024], Pcl1[:])
                nc.gpsimd.memset(XT1[D : D + 1, :], 1.0)
                nc.vector.tensor_copy(XT2[0:D, 0:512], Pue0[0:D, :])
                nc.vector.tensor_copy(XT2[0:D, 512:1024], Pue1[0:D, :])
                nc.gpsimd.memset(XT2[D : D + 1, :], 1.0)
                if not last:
                    XT3 = sbE.tile([D + 1, R], f32, tag="xt3")
                    nc.vector.tensor_copy(XT3[0:D, 0:512], Pue0[D:P, :])
                    nc.vector.tensor_copy(XT3[0:D, 512:1024], Pue1[D:P, :])
                    nc.gpsimd.memset(XT3[D : D + 1, :], 1.0)

                Pn1 = ps.tile([P, 8, D], f32, tag="pn1")
                Pn2 = ps.tile([P, 8, D], f32, tag="pn2")
                if not last:
                    Pnue = ps.tile([P, 8, D], f32, tag="pnue")
                for jj in range(JT):
                    sl = slice(jj * P, (jj + 1) * P)
                    nc.tensor.matmul(Pn1[:, jj, :], XT1[:, sl], w1x[:, l, :], start=True, stop=True)
                    nc.tensor.matmul(Pn2[:, jj, :], XT2[:, sl], w2x[:, l, :], start=True, stop=True)
                    if not last:
                        nc.tensor.matmul(Pnue[:, jj, :], XT3[:, sl], w3x[:, l, :], start=True, stop=True)

                t1 = sbE.tile([P, 8, D], f32, tag="t1")
                t2 = sbE.tile([P, 8, D], f32, tag="t2")
                nc.scalar.activation(t1[:], Pn1[:], RELU)
                nc.scalar.activation(t2[:], Pn2[:], RELU)

                if not last:
                    Epad = sbE.tile([P, JT, 2 * D], f32, tag="epad")
                    nc.scalar.activation(Epad[:, :, 0:D], Pnue[:], RELU)
                    nc.vector.tensor_tensor(Epad[:, :, D : 2 * D], t1[:], t2[:], ADD)
                    for jj in range(JT):
                        nc.sync.dma_start(out=Lg[jj], in_=Epad[:, jj, :])
                    nc.gpsimd.collective_compute(
                        "AllGather",
                        BYPASS,
                        replica_groups=groups,
                        ins=[Lg[:].opt()],
                        outs=[Gg[:].opt()],
                    )
                    nc.sync.dma_start(
                        out=Hbuf[:], in_=Gg[:].rearrange("j p c -> p j c")
                    )
                else:
                    # H_cl3 block -> column sum -> AllReduce -> head MLP
                    hs = sbE.tile([P, JT, D], f32, tag="hs")
                    nc.vector.tensor_tensor(hs[:], t1[:], t2[:], ADD)
                    Ppool = ps.tile([D, 1], f32, tag="pooled")
                    for jj in range(JT):
                        nc.tensor.matmul(
                            Ppool[:], hs[:, jj, :], ones_mv[:],
                            start=(jj == 0), stop=(jj == JT - 1),
                        )
                    pl_s = sbE.tile([D, 1], f32, tag="pl")
                    nc.vector.tensor_copy(pl_s[:], Ppool[:])
                    nc.sync.dma_start(out=prd_l[:], in_=pl_s[:])
                    nc.gpsimd.collective_compute(
                        "AllReduce",
                        ADD,
                        replica_groups=groups,
                        ins=[prd_l[:].opt()],
                        outs=[prd_s[:].opt()],
                    )
                    pvec = sbE.tile([D + 1, 1], f32, tag="pvec")
                    nc.sync.dma_start(out=pvec[0:D, :], in_=prd_s[:])
                    nc.gpsimd.memset(pvec[D : D + 1, :], 1.0)
                    Pz = ps.tile([D, 1], f32, tag="pooled")
                    nc.tensor.matmul(Pz[:], q1x[:], pvec[:], start=True, stop=True)
                    zt = sbE.tile([D + 1, 1], f32, tag="zt")
                    nc.scalar.activation(zt[0:D, :], Pz[:], RELU)
                    nc.gpsimd.memset(zt[D : D + 1, :], 1.0)
                    Po = ps.tile([1, 1], f32, tag="pooled")
                    nc.tensor.matmul(Po[:], q2x[:], zt[:], start=True, stop=True)
                    o_s = sbE.tile([1, 1], f32, tag="os")
                    nc.vector.tensor_copy(o_s[:], Po[:])
                    nc.sync.dma_start(out=out_d[:], in_=o_s[:])

    nc.compile()
    return nc


def _get_module():
    global _CACHED
    if _CACHED is None:
        _CACHED = _build_module()
    return _CACHED


def prep_in_maps(inputs):
    f = np.float32
    A_cl = np.asarray(inputs["A_cl"], f)
    A_ue = np.asarray(inputs["A_ue"], f)
    ones_row = np.ones((1, N), f)
    X1T = np.ascontiguousarray(np.vstack([np.asarray(inputs["X_cl_1"], f).T, ones_row]))
    X2T = np.ascontiguousarray(np.vstack([np.asarray(inputs["X_cl_2"], f).T, ones_row]))
    XueT = np.ascontiguousarray(np.vstack([np.asarray(inputs["X_ue"], f).T, ones_row]))

    def wx0(w, b):
        return np.ascontiguousarray(np.vstack([np.asarray(w, f), np.asarray(b, f)[None, :]]))

    def wx(w, b):
        # [3, D, D] + [3, D] -> [D+1, 3, D]
        w = np.asarray(w, f)
        b = np.asarray(b, f)
        stk = np.stack([np.vstack([w[i], b[i][None, :]]) for i in range(3)], axis=1)
        return np.ascontiguousarray(stk)

    common = {
        "X1T": X1T,
        "X2T": X2T,
        "XueT": XueT,
        "w10": wx0(inputs["W1_w0"], inputs["W1_b0"]),
        "w20": wx0(inputs["W2_w0"], inputs["W2_b0"]),
        "w30": wx0(inputs["W3_w0"], inputs["W3_b0"]),
        "w1x": wx(inputs["W1_w"], inputs["W1_b"]),
        "w2x": wx(inputs["W2_w"], inputs["W2_b"]),
        "w3x": wx(inputs["W3_w"], inputs["W3_b"]),
        "q1x": wx0(inputs["Q_w1"], inputs["Q_b1"]),
        "q2x": np.ascontiguousarray(
            np.vstack([np.asarray(inputs["Q_w2"], f), np.asarray(inputs["Q_b2"], f)[None, :]])
        ),
    }

    in_maps = []
    for c in range(M):
        rs = slice(c * R, (c + 1) * R)
        m = dict(common)
        m["AclT"] = np.ascontiguousarray(A_cl[rs, :].T)
        m["AueT"] = np.ascontiguousarray(A_ue[rs, :].T)
        in_maps.append(m)
    return in_maps


def kernel(**inputs):
    global LAST_EXEC_NS, LAST_PROFILE
    nc = _get_module()
    from concourse.bass_utils import run_bass_kernel_spmd

    in_maps = prep_in_maps(inputs)
    res = run_bass_kernel_spmd(nc, in_maps, core_ids=list(range(M)), trace=False)
    LAST_EXEC_NS = res.exec_time_ns
    LAST_PROFILE = res.profile_json
    return np.asarray(res.results[0]["out"], np.float32)
